# revision 14
# baseline (speedup 1.0000x reference)
"""Multi-head causal attention with RoPE on 8 Trainium2 NeuronCores.

Reference computation (B=2, T=2048, C=1024, H=16, Dh=64, fp32):
    qkv = x @ w_qkv + b_qkv ; split q,k,v ; RoPE(q), RoPE(k)
    attn = softmax_causal(q k^T / sqrt(Dh)) @ v ; out = attn @ w_proj + b_proj

Sharding: core c = b*4 + g handles batch b and head group g (heads 4g..4g+3).
Data-parallel over batch, tensor-parallel over heads (w_qkv column-split,
w_proj row-split).  Each core emits a partial [T, C] projection output; the
host sums the 4 per-batch partials and adds b_proj.

Per-core kernel.  Heavy matmuls in bf16 (1 PE cycle/column), fp32 PSUM
accumulation.  Design notes driven by the TRN2 cost model:
  - The PE only reaches its 2.4 GHz p-state after 3us of gap-free
    execution; every dependency stall resets it to 1.2 GHz.  So the
    instruction stream is ordered to keep the PE dense: QKV chunks start
    as soon as their DMA lands, attention S->exp->PV is pipelined with a
    deep pending queue, and nothing on the PE ever waits for ACT tables.
  - Input DMAs are split into ~128KB pieces round-robined over three
    engine queues in compute order (one dma_start binds one of the 16 hw
    queues at ~22GB/s, so parallelism requires many calls).
  - x^T is pre-transposed on the host; QKV runs weights-stationary
    producing Q^T/K^T in [head_dim, T] layout; V in [T, head_dim].
  - qkv bias: Q/K bias is per-partition in the Q^T layout, folded into
    the RoPE multiplies via scalar_tensor_tensor((pq + b) * cos); V bias
    is a partition-replicated tile folded into the PSUM->SBUF copy.
  - RoPE: rope(q) = (q+b)*cos + shift32((q+b)*sin_perm), the partition
    shift done with a constant 128x128 0/1 permutation matmul.
  - attention per head: S^T tile = K_j Q^T (scores transposed so the
    softmax sum direction matches the PE contraction), two chunks share a
    2-bank PSUM supertile so ACT exps run at [128, ~1024] granularity
    (halves ACT per-instruction overhead), exp fuses the 1/sqrt(Dh)
    scale (no max subtraction: scores are ~N(0,1); fp32 exp cannot
    overflow), causal via narrowing each k-tile's q-range plus one
    triangular -400-add matmul on diagonal 128x128 blocks.
  - V is augmented with a ones column so the PV matmul also emits the
    softmax denominator; 1/denom via DVE reciprocal_approx_fast (no
    Ln/Exp ACT table thrash -- the exp table is loaded exactly once),
    broadcast across partitions with a rank-1 fp32r matmul.
  - projection: per head-pair stationary attn^T tiles vs w_proj rows;
    output stored bf16 (host accumulates the 4 partials in fp32).
"""

import numpy as np
import ml_dtypes

import concourse.bacc as bacc
import concourse.bass as bass
import concourse.mybir as mybir
from concourse.tile import TileContext
from concourse.bass_utils import run_bass_kernel_spmd

F32 = mybir.dt.float32
F32R = mybir.dt.float32r
BF16 = mybir.dt.bfloat16
NPBF16 = np.dtype(ml_dtypes.bfloat16)

B, T, C = 2, 2048, 1024
H, DH = 16, 64
GH = 4  # heads per core
N_CORES = 8
NCHUNK = C // 128  # 8 contraction chunks
NT = T // 128  # 16 token tiles
NSPAN = T // 512  # 4 query spans
QK_COLS = 2 * GH * DH  # 512 = q cols (256) + k cols (256)
VA = GH * (DH + 1)  # 260 = v cols augmented with ones column per head
EXP = mybir.ActivationFunctionType.Exp
ADD = mybir.AluOpType.add
MULT = mybir.AluOpType.mult


def _build():
    nc = bacc.Bacc("TRN2", target_bir_lowering=False, debug=False, num_devices=N_CORES)

    xT = nc.dram_tensor("xT", [C, T], BF16, kind="ExternalInput")
    wqk = nc.dram_tensor("wqk", [C, QK_COLS], BF16, kind="ExternalInput")
    wv = nc.dram_tensor("wv", [C, VA], BF16, kind="ExternalInput")
    bqcol_d = nc.dram_tensor("bqcol", [128, 4], F32, kind="ExternalInput")
    bv128_d = nc.dram_tensor("bv128", [128, VA], F32, kind="ExternalInput")
    cos_d = nc.dram_tensor("cosT", [128, T], BF16, kind="ExternalInput")
    sinp_d = nc.dram_tensor("sinTp", [128, T], BF16, kind="ExternalInput")
    perm_d = nc.dram_tensor("perm", [128, 128], BF16, kind="ExternalInput")
    tri_d = nc.dram_tensor("tri01", [128, 128], BF16, kind="ExternalInput")
    wproj_d = nc.dram_tensor("wproj", [2, 128, C], BF16, kind="ExternalInput")
    out_d = nc.dram_tensor("out", [T, C], BF16, kind="ExternalOutput")

    with TileContext(nc) as tc:
        with tc.tile_pool(name="persist", bufs=1) as pers:
            # DMA issue: one dma_start binds one hw queue, so split inputs
            # into ~128KB pieces and round-robin issuing engines in the
            # order phase 1 consumes them.
            dmae = [nc.sync, nc.gpsimd, nc.scalar]
            qi = [0]

            def dma(dst, src):
                dmae[qi[0] % len(dmae)].dma_start(out=dst, in_=src)
                qi[0] += 1

            ones_ff = pers.tile([128, 64], F32, tag="ones_ff")
            nc.vector.memset(ones_ff, 1.0)
            ones_r = pers.tile([128, 64], F32R, tag="ones_r")
            nc.vector.tensor_copy(ones_r, ones_ff)

            bv128_sb = pers.tile([128, VA], F32, tag="bv128")
            dma(bv128_sb, bv128_d[:, :])
            bqcol_sb = pers.tile([128, 4], F32, tag="bqcol")
            dma(bqcol_sb, bqcol_d[:, :])

            perm_sb = pers.tile([128, 128], BF16, tag="perm")
            tri_sb = pers.tile([128, 128], BF16, tag="tri01")
            wproj_sb = []
            for p in range(2):
                t = pers.tile([128, C], BF16, tag="wproj", bufs=2, name=f"wproj{p}")
                wproj_sb.append(t)

            def dma2(dst, src_ap):
                # split a [128, N] transfer across two hw queues
                dma(dst[0:64, :], src_ap[0:64, :])
                dma(dst[64:128, :], src_ap[64:128, :])

            # Outputs of phase 1 (live into phase 2/3)
            qkt = []  # 4 tiles [128, T]: Q heads(0,1), Q(2,3), K(0,1), K(2,3)
            for i in range(4):
                t = pers.tile([128, T], BF16, tag="qkt", bufs=4, name=f"qkt{i}")
                qkt.append(t)
            vaug = []  # 16 tiles [128, VA], k-tile-major natural layout V
            for j in range(NT):
                t = pers.tile([128, VA], BF16, tag="vaug", bufs=NT, name=f"vaug{j}")
                vaug.append(t)
            attn = []  # 2 tiles [128, T]: normalized attn^T for head pairs
            for p in range(2):
                t = pers.tile([128, T], BF16, tag="attn", bufs=2, name=f"attn{p}")
                attn.append(t)

            # ------- Phase 1+2 merged: V first, then QK || attention -------
            # SBUF that outlives the V sub-phase lives in p12; V's weights
            # and PSUM live in a short inner pool whose banks are recycled
            # into the attention pools.
            with tc.tile_pool(name="p12", bufs=1) as p12:
                xsp = [[None] * NSPAN for _ in range(NCHUNK)]
                cos_sp = [None] * NSPAN
                sinp_sp = [None] * NSPAN

                def load_span(sp):
                    for kc in range(NCHUNK):
                        t = p12.tile(
                            [128, 512], BF16, tag="xsp",
                            bufs=NCHUNK * NSPAN, name=f"x{kc}_{sp}",
                        )
                        dma2(t, xT[128 * kc : 128 * (kc + 1), 512 * sp : 512 * (sp + 1)])
                        xsp[kc][sp] = t

                def load_trig(sp):
                    tcs = p12.tile([128, 512], BF16, tag="cos", bufs=NSPAN, name=f"cos{sp}")
                    dma2(tcs, cos_d[:, 512 * sp : 512 * (sp + 1)])
                    cos_sp[sp] = tcs
                    tsn = p12.tile([128, 512], BF16, tag="sinp", bufs=NSPAN, name=f"sin{sp}")
                    dma2(tsn, sinp_d[:, 512 * sp : 512 * (sp + 1)])
                    sinp_sp[sp] = tsn

                wqk_t = [None] * NCHUNK

                # ---- V sub-phase (inner pool; PSUM banks recycled after) ----
                with (
                    tc.tile_pool(name="pv", bufs=1) as pvp,
                    tc.tile_pool(name="pvps", bufs=1, space="PSUM") as vps,
                ):
                    wv_t = []
                    for kc in range(NCHUNK):
                        t = pvp.tile([128, VA], BF16, tag="wv", bufs=NCHUNK, name=f"wv{kc}")
                        dma(t, wv[128 * kc : 128 * (kc + 1), :])
                        wv_t.append(t)
                    load_span(0)
                    load_span(1)
                    load_span(2)
                    load_span(3)
                    for kc in range(NCHUNK):
                        t = p12.tile(
                            [128, QK_COLS], BF16, tag="wqk", bufs=NCHUNK, name=f"wqk{kc}"
                        )
                        dma2(t, wqk[128 * kc : 128 * (kc + 1), :])
                        wqk_t[kc] = t
                    dma(perm_sb, perm_d[:, :])
                    dma(tri_sb, tri_d[:, :])
                    load_trig(0)
                    load_trig(1)
                    load_trig(2)
                    load_trig(3)
                    for p in range(2):
                        dma2(wproj_sb[p], wproj_d[p, :, :])

                    for it in range(NT):
                        # V natural layout: [128 tok, VA cols]; bias (incl
                        # the ones columns) fused into the PSUM->SBUF copy.
                        sp, li = divmod(it, 4)
                        pv = vps.tile([128, VA], F32, tag="psv", bufs=2, name="psv")
                        xs = slice(128 * li, 128 * (li + 1))
                        for kc in range(NCHUNK):
                            nc.tensor.matmul(
                                pv, xsp[kc][sp][:, xs], wv_t[kc],
                                start=(kc == 0), stop=(kc == NCHUNK - 1),
                            )
                        nc.vector.tensor_add(vaug[it], pv, bv128_sb)

                # ---- merged QK projection + causal attention stream ----
                with tc.tile_pool(name="p2ps", bufs=1, space="PSUM") as p2ps:
                    from collections import deque

                    pending = deque()   # PV chunks awaiting emission
                    norm_q = deque()    # deferred normalize tails
                    avail = deque()     # stream pair emitters ready to run
                    pvps = {}

                    def qk_unit(ct, sp):
                        # Q^T/K^T col-tile for one span, bias + RoPE fused:
                        # qkt = (pq+b)*cos + perm @ ((pq+b)*sin_perm).
                        # The perm matmul overwrites pq in place (bank reuse).
                        ss = slice(512 * sp, 512 * (sp + 1))
                        cs = slice(128 * ct, 128 * (ct + 1))
                        bq = bqcol_sb[:, ct : ct + 1]
                        pq = p2ps.tile([128, 512], F32, tag="ps1", bufs=1, name="ps1")
                        for kc in range(NCHUNK):
                            nc.tensor.matmul(
                                pq, wqk_t[kc][:, cs], xsp[kc][sp],
                                start=(kc == 0), stop=(kc == NCHUNK - 1),
                            )
                        t2 = p12.tile([128, 512], BF16, tag="t2", bufs=2, name="t2")
                        nc.vector.scalar_tensor_tensor(t2, pq, bq, sinp_sp[sp], ADD, MULT)
                        nc.vector.scalar_tensor_tensor(
                            qkt[ct][:, ss], pq, bq, cos_sp[sp], ADD, MULT
                        )
                        nc.tensor.matmul(pq, perm_sb, t2, start=True, stop=True)
                        nc.vector.tensor_add(qkt[ct][:, ss], qkt[ct][:, ss], pq)

                    def normalize_tail(h, s):
                        # deferred one pair so the rb matmul never stalls
                        # the PE on the d_r DVE copy
                        ct = h // 2
                        po = (h % 2) * 64
                        pv, d_r = pvps.pop((h, s))
                        rb = p2ps.tile([64, 512], F32, tag="psrb", bufs=1, name="psrb")
                        nc.tensor.matmul(
                            rb, ones_r[64:65, :], d_r[64:65, :], start=True, stop=True
                        )
                        rbs = p12.tile([64, 512], F32, tag="rbs", bufs=2, name="rbs")
                        nc.vector.reciprocal_approx_fast(out=rbs[:, :], in_=rb[0:64, :])
                        nc.vector.tensor_mul(
                            attn[ct][po : po + 64, 512 * s : 512 * (s + 1)],
                            pv[0:64, :],
                            rbs,
                        )

                    def emit_pv(item):
                        h, j, s, qo, w, et, eo = item
                        if (h, s) not in pvps:
                            pvps[(h, s)] = [
                                p2ps.tile(
                                    [65, 512], F32, tag="pspv", bufs=2,
                                    name=f"pspv{h}_{s}",
                                ),
                                None,
                            ]
                        pv = pvps[(h, s)][0]
                        last = j == 4 * s + 3
                        nc.tensor.matmul(
                            pv[:, qo : qo + w],
                            vaug[j][:, 65 * h : 65 * (h + 1)],
                            et[:, eo : eo + w],
                            start=(j == 0),
                            stop=last,
                        )
                        if last:
                            # broadcast the raw denominator across partitions
                            # later with a rank-1 fp32r matmul; reciprocal is
                            # taken on the [64,512] broadcast (custom DVE ops
                            # need base partition 0, and it fuses the
                            # PSUM->SBUF copy)
                            d_r = p12.tile(
                                [65, 512], F32R, tag="d_r", bufs=2, name="d_r"
                            )
                            nc.vector.tensor_copy(d_r[64:65, :], pv[64:65, :])
                            pvps[(h, s)][1] = d_r
                            norm_q.append((h, s))

                    def make_pair(h, s, pair):
                        ct = h // 2
                        po = (h % 2) * 64
                        qt, kt = qkt[ct], qkt[2 + ct]

                        def emit():
                            # one deferred normalize tail per pair slot
                            if norm_q:
                                normalize_tail(*norm_q.popleft())
                            # two S^T chunks share a 2-bank PSUM supertile so
                            # one ACT exp covers both (less ACT overhead)
                            sup = p2ps.tile(
                                [128, 1024], F32, tag="pss", bufs=2, name="pss"
                            )
                            off = 0
                            offs = []
                            for j, qo, w in pair:
                                nc.tensor.matmul(
                                    sup[:, off : off + w],
                                    kt[po : po + 64, 128 * j : 128 * (j + 1)],
                                    qt[po : po + 64, 512 * s + qo : 512 * s + qo + w],
                                    start=True,
                                    stop=True,
                                )
                                offs.append(off)
                                off += w
                            et = p12.tile([128, 1024], BF16, tag="et", bufs=8, name="et")
                            nc.scalar.activation(
                                out=et[:, 0:off], in_=sup[:, 0:off], func=EXP,
                                scale=0.125,
                            )
                            for (j, qo, w), eo in zip(pair, offs):
                                if j >= 4 * s:
                                    # causal mask on the diagonal 128-block:
                                    # zero probs where k > q (DVE, off the
                                    # PE critical path)
                                    nc.vector.tensor_mul(
                                        et[:, eo : eo + 128],
                                        et[:, eo : eo + 128],
                                        tri_sb,
                                    )
                            for (j, qo, w), eo in zip(pair, offs):
                                pending.append((h, j, s, qo, w, et, eo))
                            while len(pending) > 6:
                                emit_pv(pending.popleft())

                        return emit

                    def enqueue_heads(s, heads):
                        for h in heads:
                            chunks = []  # (j, q-offset within span, width)
                            for j in range(4 * s + 4):
                                q0 = max(512 * s, 128 * j)
                                chunks.append((j, q0 - 512 * s, 512 * (s + 1) - q0))
                            for a in range(0, len(chunks), 2):
                                avail.append(
                                    make_pair(h, s, (chunks[a], chunks[a + 1]))
                                )

                    def emit_pairs(n):
                        while n > 0 and avail:
                            avail.popleft()()
                            n -= 1

                    # K before Q for the heads that stream first; 3 stream
                    # pairs between QK units keeps the PE dense while the
                    # rope DVE chain drains (ps1 has a single buffer)
                    for sp in range(NSPAN):
                        for ct in (2, 0, 3, 1):
                            qk_unit(ct, sp)
                            if ct == 0:
                                enqueue_heads(sp, (0, 1))
                            elif ct == 1:
                                enqueue_heads(sp, (2, 3))
                            emit_pairs(3 if len(avail) > 10 else 2)
                    while avail:
                        avail.popleft()()
                    while pending or norm_q:
                        if pending:
                            emit_pv(pending.popleft())
                        if norm_q and (len(pending) < 4):
                            normalize_tail(*norm_q.popleft())

            # ---------------- Phase 3: output projection ------------------
            with (
                tc.tile_pool(name="p3", bufs=1) as p3,
                tc.tile_pool(name="p3ps", bufs=1, space="PSUM") as p3ps,
            ):
                oq = [0]
                oeng = [nc.sync, nc.gpsimd]
                for it in range(NT):
                    ts = slice(128 * it, 128 * (it + 1))
                    pp = p3ps.tile([128, C], F32, tag="psproj", bufs=3, name="psproj")
                    for p in range(2):
                        for nh in range(2):
                            ns = slice(512 * nh, 512 * (nh + 1))
                            nc.tensor.matmul(
                                pp[:, ns],
                                attn[p][:, ts],
                                wproj_sb[p][:, ns],
                                start=(p == 0),
                                stop=(p == 1),
                            )
                    ob = p3.tile([128, C], BF16, tag="ob", bufs=4, name="ob")
                    # split PSUM->SBUF copy across both engines per tile
                    nc.scalar.copy(ob[:, 0:512], pp[:, 0:512])
                    nc.vector.tensor_copy(ob[:, 512:1024], pp[:, 512:1024])
                    # split the 256KB store across 4 queue slots
                    for qt4 in range(4):
                        hs = slice(32 * qt4, 32 * (qt4 + 1))
                        ds = slice(128 * it + 32 * qt4, 128 * it + 32 * (qt4 + 1))
                        oeng[oq[0] % 2].dma_start(out=out_d[ds, :], in_=ob[hs, :])
                        oq[0] += 1

    nc.compile()
    return nc


_NC = None


def _get_nc():
    global _NC
    if _NC is None:
        _NC = _build()
    return _NC


def _rope_tables():
    theta = (10000.0 ** (-np.arange(0, DH, 2, dtype=np.float32) / DH)).astype(
        np.float32
    )
    t = np.arange(T, dtype=np.float32)
    sinusoid = np.outer(t, theta).astype(np.float32)  # [T, DH/2]
    sin = np.concatenate([np.sin(sinusoid), np.sin(sinusoid)], axis=1)  # [T, DH]
    cos = np.concatenate([np.cos(sinusoid), np.cos(sinusoid)], axis=1)
    cosT = cos.T  # [DH, T]
    sinT = sin.T
    # sin_perm[e] = sin[(e+32) % 64]
    idx = (np.arange(DH) + 32) % DH
    sinTp = sinT[idx]
    cos2 = np.ascontiguousarray(np.concatenate([cosT, cosT], axis=0))  # [128, T]
    sinp2 = np.ascontiguousarray(np.concatenate([sinTp, sinTp], axis=0))
    return cos2, sinp2


def _perm_matrix():
    p = np.zeros((128, 128), dtype=np.float32)
    for m in range(128):
        blk = m // 64
        k = blk * 64 + (m % 64 + 32) % 64
        p[k, m] = 1.0
    return p


def _tri01():
    # tri01[k, q] = 1 where k <= q (upper triangular incl diagonal):
    # multiplies the exp'd diagonal 128-block, zeroing future keys
    return np.triu(np.ones((128, 128), dtype=np.float32))


def _bf(a):
    return np.ascontiguousarray(np.asarray(a, dtype=np.float32).astype(NPBF16))


def _prepare_in_maps(x, w_qkv, b_qkv, w_proj):
    x = np.asarray(x, dtype=np.float32)
    w_qkv = np.asarray(w_qkv, dtype=np.float32)
    b_qkv = np.asarray(b_qkv, dtype=np.float32)
    w_proj = np.asarray(w_proj, dtype=np.float32)

    cos2, sinp2 = _rope_tables()
    cos2, sinp2 = _bf(cos2), _bf(sinp2)
    perm = _bf(_perm_matrix())
    tri01 = _bf(_tri01())
    xTs = [_bf(x[b].T) for b in range(B)]

    in_maps = []
    for c in range(N_CORES):
        b, g = divmod(c, 4)
        h0 = g * GH  # first head of the group
        qcols = w_qkv[:, h0 * DH : (h0 + GH) * DH]
        kcols = w_qkv[:, C + h0 * DH : C + (h0 + GH) * DH]
        wqk = _bf(np.concatenate([qcols, kcols], axis=1))
        wv = np.zeros((C, VA), dtype=np.float32)
        bv = np.zeros((1, VA), dtype=np.float32)
        for j in range(GH):
            src = 2 * C + (h0 + j) * DH
            wv[:, j * 65 : j * 65 + DH] = w_qkv[:, src : src + DH]
            bv[0, j * 65 : j * 65 + DH] = b_qkv[src : src + DH]
            bv[0, j * 65 + DH] = 1.0
        bv128 = np.ascontiguousarray(np.broadcast_to(bv, (128, VA)).astype(np.float32))
        bqk = np.concatenate(
            [b_qkv[h0 * DH : (h0 + GH) * DH], b_qkv[C + h0 * DH : C + (h0 + GH) * DH]]
        ).astype(np.float32)
        bqcol = np.ascontiguousarray(bqk.reshape(4, 128).T)  # [128, 4], col ct
        wproj = np.stack(
            [w_proj[(h0 + 2 * p) * DH : (h0 + 2 * p + 2) * DH, :] for p in range(2)]
        )
        in_maps.append(
            {
                "xT": xTs[b],
                "wqk": wqk,
                "wv": _bf(wv),
                "bqcol": bqcol,
                "bv128": bv128,
                "cosT": cos2,
                "sinTp": sinp2,
                "perm": perm,
                "tri01": tri01,
                "wproj": _bf(wproj),
            }
        )
    return in_maps


def run(x, w_qkv, b_qkv, w_proj, b_proj, trace=False, tmpdir=None):
    nc = _get_nc()
    in_maps = _prepare_in_maps(x, w_qkv, b_qkv, w_proj)
    res = run_bass_kernel_spmd(
        nc, in_maps, list(range(N_CORES)), trace=trace, tmpdir=tmpdir
    )
    b_proj = np.asarray(b_proj, dtype=np.float32)
    out = np.empty((B, T, C), dtype=np.float32)
    for b in range(B):
        acc = res.results[4 * b]["out"].astype(np.float32)
        for g in range(1, 4):
            acc = acc + res.results[4 * b + g]["out"].astype(np.float32)
        out[b] = acc + b_proj
    return out, res


def kernel(x, w_qkv, b_qkv, w_proj, b_proj):
    out, _ = run(x, w_qkv, b_qkv, w_proj, b_proj, trace=False)
    return out
